# revision 31
# baseline (speedup 1.0000x reference)
"""MultiHeadLatentAttention prefill kernel for 8 Trainium2 NeuronCores.

Reference computation (B=2, L=2048, D=4096, DZ=1024, H=16, HD=64):
    z      = x @ W_latent
    q_lat  = x @ W_q_down + b_q_down
    query  = concat([q_lat @ W_q_up, rope(q_lat @ W_x_rope)], -1) -> heads of 128
    key    = concat([z @ W_k_up,     rope(x @ W_k_rope)], -1)     -> heads of 128
    value  = (z @ W_v_up) -> heads of 64
    scores = mask(q @ k^T / sqrt(128))          (causal, -1e6 above diag)
    attn   = softmax(scores)
    head_out = scores @ v                        (NB: masked *scores*, not attn)
    out    = head_out @ W_o
Returns (out, z, key_rope, attn_weights, scores).

Head split subtlety: concat-then-view means heads 0-7 use only the "nope"
channels (q_lat@W_q_up vs z@W_k_up) and heads 8-15 use only the "rope"
channels (rope(q_lat@W_x_rope) vs rope(x@W_k_rope)), each with d_k=128.

Sharding: launch A is token-parallel (512 tokens/core) computing all
projections in d-major ("transposed") layouts; launch B is (batch x
head-group)-parallel (4 heads/core) computing scores/softmax/head_out;
launch C is token-parallel for the output projection.

Hardware constraint that shapes this code: a fp32/f32r Matmult lowers to an
LW-struct instruction that holds at most ONE semaphore wait, so every
matmul operand (and every PSUM-WAR releaser) must be produced on a single
sem domain - the vector engine. DMA-loaded matmul inputs therefore go
through an in-place vector "rounding copy" (which also satisfies walrus's
f32r-rounded-producer rule), and matmul-feeding masking is done with
vector mask-mult-add against constant tiles rather than gpsimd
affine_select.
"""

import math
import os
from contextlib import ExitStack

import numpy as np

import concourse.bass as bass
import concourse.mybir as mybir
import concourse.tile as tile
from concourse import bacc

B, L, D = 2, 2048, 4096
DZ, H, HD = 1024, 16, 64
NC = 8
T = (B * L) // NC  # tokens per core in launches A/C = 512
SCALE = 1.0 / math.sqrt(128.0)
NEG = -1000000.0

f32 = mybir.dt.float32
f32r = mybir.dt.float32r
AF = mybir.ActivationFunctionType
ALU = mybir.AluOpType


def _r(ap):
    return ap.bitcast(f32r)


# ---------------------------------------------------------------------------
# Launch A: per-core token shard -> all projections, d-major outputs.
# ---------------------------------------------------------------------------

def build_launch_a():
    nc = bacc.Bacc("TRN2", target_bir_lowering=False)
    xT = nc.dram_tensor("xT", [D, T], f32r, kind="ExternalInput")
    wlat = nc.dram_tensor("wlat", [D, DZ], f32r, kind="ExternalInput")
    wqd = nc.dram_tensor("wqd", [D, DZ], f32r, kind="ExternalInput")
    bqd = nc.dram_tensor("bqd", [DZ], f32, kind="ExternalInput")
    wqu = nc.dram_tensor("wqu", [DZ, H * HD], f32r, kind="ExternalInput")
    wku = nc.dram_tensor("wku", [DZ, H * HD], f32r, kind="ExternalInput")
    wvu = nc.dram_tensor("wvu", [DZ, H * HD], f32r, kind="ExternalInput")
    wxr = nc.dram_tensor("wxr", [DZ, H * HD], f32r, kind="ExternalInput")
    wkr = nc.dram_tensor("wkr", [D, H * HD], f32r, kind="ExternalInput")
    # RoPE tables, [128, T]: row p holds cos/sin((l0+t) * invfreq[p % 32]).
    # cq/sq are pre-scaled by 1/sqrt(128) (query side).
    cq = nc.dram_tensor("cq", [128, T], f32, kind="ExternalInput")
    sq = nc.dram_tensor("sq", [128, T], f32, kind="ExternalInput")
    ck = nc.dram_tensor("ck", [128, T], f32, kind="ExternalInput")
    sk = nc.dram_tensor("sk", [128, T], f32, kind="ExternalInput")
    # Transposed rotate-half matrix, blockdiag([[0,-I32],[I32,0]] x2), [128,128].
    rt = nc.dram_tensor("rt", [128, 128], f32, kind="ExternalInput")

    zT_o = nc.dram_tensor("zT", [DZ, T], f32r, kind="ExternalOutput")
    qsT_o = nc.dram_tensor("qsT", [H * HD, T], f32, kind="ExternalOutput")
    knT_o = nc.dram_tensor("knT", [H * HD, T], f32, kind="ExternalOutput")
    v_o = nc.dram_tensor("v", [T, H * HD], f32, kind="ExternalOutput")
    qrT_o = nc.dram_tensor("qrT", [H * HD, T], f32, kind="ExternalOutput")
    krT_o = nc.dram_tensor("krT", [H * HD, T], f32, kind="ExternalOutput")

    NK1 = D // 128  # 32 k-chunks for stage 1
    NK2 = DZ // 128  # 8 k-chunks for stage 2
    NM = DZ // 128  # 8 m-tiles

    with ExitStack() as ctx:
        tc = ctx.enter_context(tile.TileContext(nc))
        singles = ctx.enter_context(tc.tile_pool(name="singles", bufs=1))
        wpool = ctx.enter_context(tc.tile_pool(name="wpool", bufs=3))
        opool = ctx.enter_context(tc.tile_pool(name="opool", bufs=4))
        tpool = ctx.enter_context(tc.tile_pool(name="tpool", bufs=2))
        psum = ctx.enter_context(tc.tile_pool(name="psum", bufs=2, space="PSUM"))
        rpsum = ctx.enter_context(tc.tile_pool(name="rpsum", bufs=2, space="PSUM"))

        xT_sb = singles.tile([128, NK1, T], f32r)
        xT_r = xT.rearrange("(o p) t -> p o t", p=128)
        for xc in range(4):
            nc.sync.dma_start(
                xT_sb[:, xc * (NK1 // 4):(xc + 1) * (NK1 // 4), :],
                xT_r[:, xc * (NK1 // 4):(xc + 1) * (NK1 // 4), :],
            )
        bias_sb = singles.tile([128, NM], f32)
        nc.sync.dma_start(bias_sb, bqd.rearrange("(o p) -> p o", p=128))
        cq_sb = singles.tile([128, T], f32)
        nc.sync.dma_start(cq_sb, cq[:, :])
        sq_sb = singles.tile([128, T], f32)
        nc.sync.dma_start(sq_sb, sq[:, :])
        ck_sb = singles.tile([128, T], f32)
        nc.sync.dma_start(ck_sb, ck[:, :])
        sk_sb = singles.tile([128, T], f32)
        nc.sync.dma_start(sk_sb, sk[:, :])
        rt_sb = singles.tile([128, 128], f32)
        nc.sync.dma_start(rt_sb, rt[:, :])

        zT_sb = singles.tile([128, NM, T], f32r)
        qlT_sb = singles.tile([128, NM, T], f32r)

        def rope_tile(src_ps, c_sb, s_sb, out_tile):
            """out = src*cos + (R @ src)*sin, all [128, T] (2 heads of 64)."""
            src_sb = tpool.tile([128, T], f32, tag="t_src")
            nc.vector.tensor_copy(src_sb, src_ps)
            rot_ps = rpsum.tile([128, T], f32, name="rot_ps")
            # full fp32 matmul: exact for the +-1 rotation matrix
            nc.tensor.matmul(rot_ps, rt_sb, src_sb, start=True, stop=True)
            t_rot = tpool.tile([128, T], f32, tag="t_rot")
            nc.vector.tensor_tensor(t_rot, rot_ps, s_sb, op=ALU.mult)
            t_cos = tpool.tile([128, T], f32, tag="t_cos")
            nc.vector.tensor_tensor(t_cos, src_sb, c_sb, op=ALU.mult)
            nc.vector.tensor_tensor(out_tile, t_cos, t_rot, op=ALU.add)

        # stage 1 (contract over D from xT) and stage 2 (contract over DZ).
        # (weight, k-chunks, src or None=xT, dst sbuf, dram out, bias, rope, scale)
        projs = [
            (wlat, NK1, None, zT_sb, zT_o, False, None, None),
            (wqd, NK1, None, qlT_sb, None, True, None, None),
            (wkr, NK1, None, None, krT_o, False, (ck_sb, sk_sb), None),
            (wqu, NK2, qlT_sb, None, qsT_o, False, None, SCALE),
            (wku, NK2, zT_sb, None, knT_o, False, None, None),
            (wxr, NK2, qlT_sb, None, qrT_o, False, (cq_sb, sq_sb), None),
        ]
        for w_dram, nk, src_sb, dst_sb, out_dram, use_bias, rope, scl in projs:
            if nk == NK2:
                # stage 2: whole [DZ, 1024] weight in two [128, 4, 1024]
                # halves (4 KiB contiguous runs, 2 DMAs instead of 8)
                whs = []
                for half in range(2):
                    wh = wpool.tile([128, NK1, 128], f32r, tag="wt",
                                    name=f"wh{half}")
                    wh4 = wh.rearrange("p (a b) m -> p a (b m)", b=8)
                    nc.sync.dma_start(
                        wh4[:, :4, :],
                        w_dram[half * 512:(half + 1) * 512, :].rearrange(
                            "(ko p) m -> p ko m", p=128
                        ),
                    )
                    whs.append(wh4)
            for mi in range(NM):
                if nk == NK2:
                    ps = psum.tile([128, T], f32, tag="ps_m", name="ps_m")
                    for ki in range(nk):
                        nc.tensor.matmul(
                            ps,
                            whs[ki // 4][:, ki % 4, mi * 128:(mi + 1) * 128],
                            src_sb[:, ki, :],
                            start=(ki == 0),
                            stop=(ki == nk - 1),
                        )
                else:
                    wt = wpool.tile([128, NK1, 128], f32r, tag="wt", name="wt")
                    nc.sync.dma_start(
                        wt,
                        w_dram[:, mi * 128:(mi + 1) * 128].rearrange(
                            "(ks p) m -> p ks m", p=128
                        ),
                    )
                    ps = psum.tile([128, T], f32, tag="ps_m", name="ps_m")
                    for ki in range(nk):
                        nc.tensor.matmul(
                            ps,
                            wt[:, ki, :],
                            xT_sb[:, ki, :],
                            start=(ki == 0),
                            stop=(ki == nk - 1),
                        )
                if dst_sb is not None:
                    stage = dst_sb[:, mi, :]
                else:
                    stage = opool.tile([128, T], f32, tag="stage1")
                if use_bias:
                    nc.vector.tensor_scalar_add(stage, ps, bias_sb[:, mi:mi + 1])
                elif rope is not None:
                    rope_tile(ps, rope[0], rope[1], stage)
                elif scl is not None:
                    nc.vector.tensor_scalar_mul(stage, ps, scl)
                else:
                    nc.vector.tensor_copy(stage, ps)
                if out_dram is not None:
                    nc.sync.dma_start(out_dram[mi * 128:(mi + 1) * 128, :], stage)

        # v = z @ W_v_up, token-major [T, 1024]
        for ch in range(2):
            wv = wpool.tile([128, NK1, 128], f32r, tag="wt", name="wv")
            wv4 = wv.rearrange("p (a b) m -> p a (b m)", b=4)  # [128,8,512] view
            nc.sync.dma_start(
                wv4[:, :NK2, :],
                wvu[:, ch * 512:(ch + 1) * 512].rearrange(
                    "(ko p) m -> p ko m", p=128
                ),
            )
            for ti in range(T // 128):
                ps = psum.tile([128, 512], f32, tag="ps_m", name="ps_v")
                for ki in range(NK2):
                    nc.tensor.matmul(
                        ps,
                        zT_sb[:, ki, ti * 128:(ti + 1) * 128],
                        wv4[:, ki, :],
                        start=(ki == 0),
                        stop=(ki == NK2 - 1),
                    )
                stage = opool.tile([128, 512], f32, tag="stage_v")
                nc.vector.tensor_copy(stage, ps)
                nc.sync.dma_start(
                    v_o[ti * 128:(ti + 1) * 128, ch * 512:(ch + 1) * 512], stage
                )
    nc.compile()
    return nc


# ---------------------------------------------------------------------------
# Launch B: per-core (batch, head-group of 4) -> scores, attn, head_outT.
# ---------------------------------------------------------------------------

def build_launch_b():
    nc = bacc.Bacc("TRN2", target_bir_lowering=False)
    qT = nc.dram_tensor("qT", [512, L], f32r, kind="ExternalInput")
    kT = nc.dram_tensor("kT", [512, L], f32r, kind="ExternalInput")
    vv = nc.dram_tensor("vv", [L, 256], f32r, kind="ExternalInput")
    sc_o = nc.dram_tensor("sc", [4, L, L], f32, kind="ExternalOutput")
    at_o = nc.dram_tensor("at", [4, L, L], f32, kind="ExternalOutput")
    hoT_o = nc.dram_tensor("hoT", [256, L], f32, kind="ExternalOutput")

    NQT = L // 128  # 16 q-tiles of 128
    NQB = L // 512  # 4 q-blocks of 512
    NKC = L // 128  # 16 k-chunks of 128

    with ExitStack() as ctx:
        tc = ctx.enter_context(tile.TileContext(nc))
        singles = ctx.enter_context(tc.tile_pool(name="singles", bufs=1))
        hpool = ctx.enter_context(tc.tile_pool(name="hpool", bufs=3))
        rowpool = ctx.enter_context(tc.tile_pool(name="rowpool", bufs=4))
        stpool = ctx.enter_context(tc.tile_pool(name="stpool", bufs=4))
        spool = ctx.enter_context(tc.tile_pool(name="spool", bufs=8))
        hopool = ctx.enter_context(tc.tile_pool(name="hopool", bufs=2))
        pspool = ctx.enter_context(tc.tile_pool(name="pspool", bufs=3, space="PSUM"))
        ps2pool = ctx.enter_context(tc.tile_pool(name="ps2pool", bufs=2, space="PSUM"))
        opsum = ctx.enter_context(tc.tile_pool(name="opsum", bufs=2, space="PSUM"))

        neg_reg = nc.gpsimd.to_reg(NEG)
        zero_reg = nc.gpsimd.to_reg(0.0)
        m1e6 = singles.tile([128, 512], f32)
        nc.vector.memset(m1e6, NEG)
        m1e6r = singles.tile([128, 512], f32r)
        nc.vector.tensor_copy(m1e6r, m1e6)
        v_sb = singles.tile([128, NKC, 256], f32r)
        nc.sync.dma_start(v_sb, vv.rearrange("(ko p) c -> p ko c", p=128))

        # sT-band mask pairs: for band offset j, sT tile (partition=k, free=q)
        # keeps entries with q >= k: col >= 128*j + p. masked = ps*m + c.
        ones_t = singles.tile([128, 512], f32)
        nc.vector.memset(ones_t, 1.0)
        zeros_t = singles.tile([128, 512], f32)
        nc.vector.memset(zeros_t, 0.0)
        mT = []
        cT = []
        for j in range(4):
            m_j = singles.tile([128, 512], f32, name=f"mT_{j}")
            nc.gpsimd.affine_select(
                m_j, ones_t, pattern=[[1, 512]], base=-128 * j,
                channel_multiplier=-1, compare_op=ALU.is_ge, fill=zero_reg,
            )
            c_j = singles.tile([128, 512], f32, name=f"cT_{j}")
            nc.gpsimd.affine_select(
                c_j, zeros_t, pattern=[[1, 512]], base=-128 * j,
                channel_multiplier=-1, compare_op=ALU.is_ge, fill=neg_reg,
            )
            mT.append(m_j)
            cT.append(c_j)

        def load_head(h):
            q_t = hpool.tile([128, L], f32r, tag="qT_h", name=f"qT_{h}")
            nc.sync.dma_start(q_t, qT[h * 128:(h + 1) * 128, :])
            k_t = hpool.tile([128, L], f32r, tag="kT_h", name=f"kT_{h}")
            nc.sync.dma_start(k_t, kT[h * 128:(h + 1) * 128, :])
            return q_t, k_t

        nxt = load_head(0)
        for h in range(4):
            qT_h, kT_h = nxt
            if h + 1 < 4:
                nxt = load_head(h + 1)  # prefetch before this head's DMAs

            def phase1_row(i, qT_h=qT_h, kT_h=kT_h, h=h):
                # scores row of 128 queries + softmax -> sc/at DMA
                nb = i // 4 + 1  # 512-blocks covering k <= q
                s_row = rowpool.tile([128, L], f32, tag="s_row", name="s_row")
                for kb in range(nb):
                    ps = pspool.tile([128, 512], f32, tag="ps_s", name="ps_s")
                    nc.tensor.matmul(
                        ps,
                        qT_h[:, i * 128:(i + 1) * 128],
                        kT_h[:, kb * 512:(kb + 1) * 512],
                        start=True,
                        stop=True,
                    )
                    blk = s_row[:, kb * 512:(kb + 1) * 512]
                    nc.vector.tensor_copy(blk, ps)
                    if kb == i // 4:  # diagonal block: keep iff q - k >= 0
                        nc.gpsimd.affine_select(
                            blk,
                            blk,
                            pattern=[[-1, 512]],
                            base=i * 128 - kb * 512,
                            channel_multiplier=1,
                            compare_op=ALU.is_ge,
                            fill=neg_reg,
                        )
                nc.sync.dma_start(
                    sc_o[h, i * 128:(i + 1) * 128, 0:nb * 512],
                    s_row[:, :nb * 512],
                )
                e_row = rowpool.tile([128, L], f32, tag="e_row", name="e_row")
                nc.vector.tensor_scalar_max(
                    e_row[:, :nb * 512], s_row[:, :nb * 512], -80.0
                )
                ssum = spool.tile([128, 1], f32, tag="ssum", name="ssum")
                nc.scalar.activation(
                    e_row[:, :nb * 512],
                    e_row[:, :nb * 512],
                    AF.Exp,
                    accum_out=ssum,
                )
                sinv = spool.tile([128, 1], f32, tag="sinv", name="sinv")
                nc.vector.reciprocal(sinv, ssum)
                nc.vector.tensor_scalar_mul(
                    e_row[:, :nb * 512], e_row[:, :nb * 512], sinv
                )
                return i, e_row, nb

            ho_row = hopool.tile([64, L], f32, tag="ho", name="ho_row")
            sT_store = {}

            def sT_phase(qb, qT_h=qT_h, kT_h=kT_h):
                # produce all masked-scores^T tiles for this q-block; runs
                # well before po_phase so the po matmuls never wait on DVE
                tiles = []
                for kc in range(4 * qb + 4):
                    ps2 = ps2pool.tile([128, 512], f32, tag="ps_st",
                                       name="ps_st")
                    nc.tensor.matmul(
                        ps2,
                        kT_h[:, kc * 128:(kc + 1) * 128],
                        qT_h[:, qb * 512:(qb + 1) * 512],
                        start=True,
                        stop=True,
                    )
                    sT = stpool.tile([128, 512], f32r, tag="sT", name="sT")
                    if kc >= 4 * qb:  # diagonal band: mask = ps*m + c
                        j = kc - 4 * qb
                        nc.vector.tensor_tensor(sT, ps2, mT[j], op=ALU.mult)
                        nc.vector.tensor_tensor(sT, sT, cT[j], op=ALU.add)
                    else:
                        nc.vector.tensor_copy(sT, ps2)
                    tiles.append(sT)
                sT_store[qb] = tiles

            def po_phase(qb, h=h, ho_row=ho_row):
                # head_outT q-block = (masked scores)^T-path @ v
                po = opsum.tile([64, 512], f32, tag="po", name="po")
                tiles = sT_store.pop(qb)
                for kc in range(NKC):
                    rhs = tiles[kc] if kc <= 4 * qb + 3 else m1e6r
                    nc.tensor.matmul(
                        po,
                        v_sb[:, kc, h * 64:(h + 1) * 64],
                        rhs,
                        start=(kc == 0),
                        stop=(kc == NKC - 1),
                    )
                nc.vector.tensor_copy(
                    ho_row[:, qb * 512:(qb + 1) * 512], po
                )
                if qb == NQB - 1:
                    nc.sync.dma_start(hoT_o[h * 64:(h + 1) * 64, :], ho_row)

            def at_dma(pend, h=h):
                i, e_row, nb = pend
                nc.sync.dma_start(
                    at_o[h, i * 128:(i + 1) * 128, 0:nb * 512],
                    e_row[:, :nb * 512],
                )

            # interleave so phase-2 PE work overlaps phase-1 DMA drains;
            # lag each attn DMA one row so a pending exp-chain never
            # head-of-line-blocks the next ready scores DMA
            import os as _os
            _abl = _os.environ.get("B_ABLATE", "")
            pend = None
            for i in range(NQT):
                cur = phase1_row(i)
                if pend is not None and "noat" not in _abl:
                    at_dma(pend)
                pend = cur
                if i % 4 == 3 and "nop2" not in _abl:
                    phase2_block(i // 4)
            if "noat" not in _abl:
                at_dma(pend)
    nc.compile()
    return nc


# ---------------------------------------------------------------------------
# Launch C: per-core token shard -> out = head_out @ W_o.
# ---------------------------------------------------------------------------

def build_launch_c():
    nc = bacc.Bacc("TRN2", target_bir_lowering=False)
    hoT = nc.dram_tensor("hoT", [H * HD, T], f32r, kind="ExternalInput")
    wo = nc.dram_tensor("wo", [H * HD, D], f32r, kind="ExternalInput")
    out_o = nc.dram_tensor("out", [T, D], f32, kind="ExternalOutput")

    NK = (H * HD) // 128  # 8
    with ExitStack() as ctx:
        tc = ctx.enter_context(tile.TileContext(nc))
        singles = ctx.enter_context(tc.tile_pool(name="singles", bufs=1))
        wpool = ctx.enter_context(tc.tile_pool(name="wpool", bufs=3))
        opool = ctx.enter_context(tc.tile_pool(name="opool", bufs=4))
        psum = ctx.enter_context(tc.tile_pool(name="psum", bufs=4, space="PSUM"))

        hoT_sb = singles.tile([128, NK, T], f32r)
        nc.sync.dma_start(hoT_sb, hoT.rearrange("(o p) t -> p o t", p=128))

        for nb in range(D // 512):
            wt = wpool.tile([128, NK, 512], f32r, tag="wt", name="wt")
            nc.sync.dma_start(
                wt,
                wo[:, nb * 512:(nb + 1) * 512].rearrange(
                    "(o p) d -> p o d", p=128
                ),
            )
            for ti in range(T // 128):
                ps = psum.tile([128, 512], f32, tag="ps_c")
                for ki in range(NK):
                    nc.tensor.matmul(
                        ps,
                        hoT_sb[:, ki, ti * 128:(ti + 1) * 128],
                        wt[:, ki, :],
                        start=(ki == 0),
                        stop=(ki == NK - 1),
                    )
                stage = opool.tile([128, 512], f32, tag="stage_c")
                nc.vector.tensor_copy(stage, ps)
                nc.sync.dma_start(
                    out_o[ti * 128:(ti + 1) * 128, nb * 512:(nb + 1) * 512],
                    stage,
                )
    nc.compile()
    return nc


# ---------------------------------------------------------------------------
# Host orchestration.
# ---------------------------------------------------------------------------

def _rope_tables(l0, scale):
    """[128, T] cos/sin tables mimicking the reference fp32 computation."""
    invf = (
        np.float32(1.0)
        / np.float32(10000.0)
        ** (np.arange(0, HD, 2, dtype=np.float32) / np.float32(HD))
    )  # [32]
    pos = np.arange(l0, l0 + T, dtype=np.float32)  # [T]
    p = np.arange(128)
    ang = pos[None, :] * invf[p % 32][:, None]  # [128, T], fp32 product
    ang = ang.astype(np.float32)
    c = np.cos(ang).astype(np.float32) * np.float32(scale)
    s = np.sin(ang).astype(np.float32) * np.float32(scale)
    return np.ascontiguousarray(c), np.ascontiguousarray(s)


def _rotmatT():
    r = np.zeros((128, 128), dtype=np.float32)
    for h0 in (0, 64):
        for i in range(32):
            r[h0 + i, h0 + 32 + i] = -1.0  # out[i] -= in[i+32]
            r[h0 + 32 + i, h0 + i] = 1.0  # out[i+32] += in[i]
    return np.ascontiguousarray(r.T)


_BUILT = {}


def _get(name, builder):
    if name not in _BUILT:
        _BUILT[name] = builder()
    return _BUILT[name]


_EXEC_NS = []  # per-launch exec_time_ns (if traced) of the most recent call
_WALL_S = []  # per-launch wall seconds of the most recent kernel() call


def _run(nc, in_maps, trace):
    import time as _time

    from concourse.bass_utils import run_bass_kernel_spmd

    t0 = _time.perf_counter()
    res = run_bass_kernel_spmd(
        nc, in_maps, core_ids=list(range(NC)), trace=trace
    )
    _WALL_S.append(_time.perf_counter() - t0)
    _EXEC_NS.append(res.exec_time_ns)
    return res.results


def kernel(**inputs):
    trace = bool(int(os.environ.get("KERNEL_TRACE", "0")))
    _EXEC_NS.clear()
    _WALL_S.clear()

    x = np.asarray(inputs["x"], dtype=np.float32)
    wlat = np.ascontiguousarray(np.asarray(inputs["W_latent"], np.float32))
    wqd = np.ascontiguousarray(np.asarray(inputs["W_q_down"], np.float32))
    bqd = np.ascontiguousarray(np.asarray(inputs["b_q_down"], np.float32))
    wqu = np.ascontiguousarray(np.asarray(inputs["W_q_up"], np.float32))
    wku = np.ascontiguousarray(np.asarray(inputs["W_k_up"], np.float32))
    wvu = np.ascontiguousarray(np.asarray(inputs["W_v_up"], np.float32))
    wxr = np.ascontiguousarray(np.asarray(inputs["W_x_rope"], np.float32))
    wkr = np.ascontiguousarray(np.asarray(inputs["W_k_rope"], np.float32))
    wo = np.ascontiguousarray(np.asarray(inputs["W_o"], np.float32))

    xT = np.ascontiguousarray(x.reshape(B * L, D).T)  # [D, B*L]
    rt = _rotmatT()

    # ---- launch A ----
    nc_a = _get("A", build_launch_a)
    in_maps = []
    for c in range(NC):
        l0 = (c * T) % L  # position within the batch
        cqt, sqt = _rope_tables(l0, SCALE)
        ckt, skt = _rope_tables(l0, 1.0)
        in_maps.append({
            "xT": np.ascontiguousarray(xT[:, c * T:(c + 1) * T]),
            "wlat": wlat, "wqd": wqd, "bqd": bqd,
            "wqu": wqu, "wku": wku, "wvu": wvu, "wxr": wxr, "wkr": wkr,
            "cq": cqt, "sq": sqt, "ck": ckt, "sk": skt, "rt": rt,
        })
    res_a = _run(nc_a, in_maps, trace)

    def gatherT(name):
        # per-core [C, T] -> per-batch [C, L]
        return [
            np.concatenate([res_a[4 * b + j][name] for j in range(4)], axis=1)
            for b in range(B)
        ]

    zT = gatherT("zT")
    qsT = gatherT("qsT")
    knT = gatherT("knT")
    qrT = gatherT("qrT")
    krT = gatherT("krT")
    v_full = [
        np.concatenate([res_a[4 * b + j]["v"] for j in range(4)], axis=0)
        for b in range(B)
    ]  # [L, 1024] per batch

    z = np.stack([zT[b].T for b in range(B)])  # (B, L, DZ)
    key_rope = np.stack([krT[b].T for b in range(B)])  # (B, L, H*HD)

    # ---- launch B ----
    nc_b = _get("B", build_launch_b)
    in_maps = []
    for c in range(NC):
        b, g = divmod(c, 4)
        if g < 2:  # heads 0-7: nope channels
            q_in = qsT[b][512 * g:512 * (g + 1), :]
            k_in = knT[b][512 * g:512 * (g + 1), :]
        else:  # heads 8-15: rope channels
            q_in = qrT[b][512 * (g - 2):512 * (g - 1), :]
            k_in = krT[b][512 * (g - 2):512 * (g - 1), :]
        in_maps.append({
            "qT": np.ascontiguousarray(q_in),
            "kT": np.ascontiguousarray(k_in),
            "vv": np.ascontiguousarray(v_full[b][:, 256 * g:256 * (g + 1)]),
        })
    res_b = _run(nc_b, in_maps, trace)

    scores = np.empty((B, H, L, L), dtype=np.float32)
    attn = np.zeros((B, H, L, L), dtype=np.float32)
    hoT = [np.empty((H * HD, L), dtype=np.float32) for _ in range(B)]
    for c in range(NC):
        b, g = divmod(c, 4)
        scores[b, 4 * g:4 * (g + 1)] = res_b[c]["sc"]
        attn[b, 4 * g:4 * (g + 1)] = res_b[c]["at"]
        hoT[b][256 * g:256 * (g + 1), :] = res_b[c]["hoT"]
    # masked blocks beyond each q-tile's covered range were never written on
    # device: attn is exactly 0 there (zero output buffer); scores get the
    # causal -1e6 fill here.
    for i in range(L // 128):
        k0 = (i // 4 + 1) * 512
        if k0 < L:
            scores[:, :, i * 128:(i + 1) * 128, k0:] = np.float32(NEG)
            attn[:, :, i * 128:(i + 1) * 128, k0:] = 0.0

    # ---- launch C ----
    nc_c = _get("C", build_launch_c)
    in_maps = []
    for c in range(NC):
        b = c // 4
        t0 = (c % 4) * T
        in_maps.append({
            "hoT": np.ascontiguousarray(hoT[b][:, t0:t0 + T]),
            "wo": wo,
        })
    res_c = _run(nc_c, in_maps, trace)
    out = np.empty((B, L, D), dtype=np.float32)
    for c in range(NC):
        b = c // 4
        t0 = (c % 4) * T
        out[b, t0:t0 + T, :] = res_c[c]["out"]

    return out, z, key_rope, attn, scores


# revision 36
# speedup vs baseline: 99518.1088x; 99518.1088x over previous
"""MultiHeadLatentAttention prefill kernel for 8 Trainium2 NeuronCores.

Reference computation (B=2, L=2048, D=4096, DZ=1024, H=16, HD=64):
    z      = x @ W_latent
    q_lat  = x @ W_q_down + b_q_down
    query  = concat([q_lat @ W_q_up, rope(q_lat @ W_x_rope)], -1) -> heads of 128
    key    = concat([z @ W_k_up,     rope(x @ W_k_rope)], -1)     -> heads of 128
    value  = (z @ W_v_up) -> heads of 64
    scores = mask(q @ k^T / sqrt(128))          (causal, -1e6 above diag)
    attn   = softmax(scores)
    head_out = scores @ v                        (NB: masked *scores*, not attn)
    out    = head_out @ W_o
Returns (out, z, key_rope, attn_weights, scores).

Head split subtlety: concat-then-view means heads 0-7 use only the "nope"
channels (q_lat@W_q_up vs z@W_k_up) and heads 8-15 use only the "rope"
channels (rope(q_lat@W_x_rope) vs rope(x@W_k_rope)), each with d_k=128.

Sharding: launch A is token-parallel (512 tokens/core) computing all
projections in d-major ("transposed") layouts; launch B is (batch x
head-group)-parallel (4 heads/core) computing scores/softmax/head_out;
launch C is token-parallel for the output projection.

Implementation notes: kernels are built with bacc.Bacc and .compile()d so
multi-wait instructions get lowered to EVSEM chains (walrus allows <=1
sync wait per instruction). Matmul-input tensors are declared float32r
end-to-end (walrus requires f32r matmul inputs to have f32r-typed
producers); f32r streams at 1 cycle/row vs fp32's 4 and measured ~2e-4
relative error on K=4096 contractions (~1e-9 at K=128). Fully-masked
causal blocks are never written on device: attn relies on the donated
zero output buffers, scores' -1e6 region is filled on the host.
"""

import math
import os
from contextlib import ExitStack

import numpy as np

import concourse.bass as bass
import concourse.mybir as mybir
import concourse.tile as tile
from concourse import bacc

B, L, D = 2, 2048, 4096
DZ, H, HD = 1024, 16, 64
NC = 8
T = (B * L) // NC  # tokens per core in launches A/C = 512
SCALE = 1.0 / math.sqrt(128.0)
NEG = -1000000.0

f32 = mybir.dt.float32
f32r = mybir.dt.float32r
AF = mybir.ActivationFunctionType
ALU = mybir.AluOpType


# ---------------------------------------------------------------------------
# Launch A: per-core token shard -> all projections, d-major outputs.
# ---------------------------------------------------------------------------

def build_launch_a():
    nc = bacc.Bacc("TRN2", target_bir_lowering=False)
    xT = nc.dram_tensor("xT", [D, T], f32r, kind="ExternalInput")
    wlat = nc.dram_tensor("wlat", [D, DZ], f32r, kind="ExternalInput")
    wqd = nc.dram_tensor("wqd", [D, DZ], f32r, kind="ExternalInput")
    bqd = nc.dram_tensor("bqd", [DZ], f32, kind="ExternalInput")
    wqu = nc.dram_tensor("wqu", [DZ, H * HD], f32r, kind="ExternalInput")
    wku = nc.dram_tensor("wku", [DZ, H * HD], f32r, kind="ExternalInput")
    wvu = nc.dram_tensor("wvu", [DZ, H * HD], f32r, kind="ExternalInput")
    wxr = nc.dram_tensor("wxr", [DZ, H * HD], f32r, kind="ExternalInput")
    wkr = nc.dram_tensor("wkr", [D, H * HD], f32r, kind="ExternalInput")
    # RoPE tables, [128, T]: row p holds cos/sin((l0+t) * invfreq[p % 32]).
    # cq/sq are pre-scaled by 1/sqrt(128) (query side).
    cq = nc.dram_tensor("cq", [128, T], f32, kind="ExternalInput")
    sq = nc.dram_tensor("sq", [128, T], f32, kind="ExternalInput")
    ck = nc.dram_tensor("ck", [128, T], f32, kind="ExternalInput")
    sk = nc.dram_tensor("sk", [128, T], f32, kind="ExternalInput")
    # Transposed rotate-half matrix, blockdiag([[0,-I32],[I32,0]] x2), [128,128].
    rt = nc.dram_tensor("rt", [128, 128], f32, kind="ExternalInput")

    zT_o = nc.dram_tensor("zT", [DZ, T], f32r, kind="ExternalOutput")
    qsT_o = nc.dram_tensor("qsT", [H * HD, T], f32, kind="ExternalOutput")
    knT_o = nc.dram_tensor("knT", [H * HD, T], f32, kind="ExternalOutput")
    v_o = nc.dram_tensor("v", [T, H * HD], f32, kind="ExternalOutput")
    qrT_o = nc.dram_tensor("qrT", [H * HD, T], f32, kind="ExternalOutput")
    krT_o = nc.dram_tensor("krT", [H * HD, T], f32, kind="ExternalOutput")

    NK1 = D // 128  # 32 k-chunks for stage 1
    NK2 = DZ // 128  # 8 k-chunks for stage 2
    NM = DZ // 128  # 8 m-tiles

    with ExitStack() as ctx:
        tc = ctx.enter_context(tile.TileContext(nc))
        singles = ctx.enter_context(tc.tile_pool(name="singles", bufs=1))
        wpool = ctx.enter_context(tc.tile_pool(name="wpool", bufs=3))
        opool = ctx.enter_context(tc.tile_pool(name="opool", bufs=4))
        tpool = ctx.enter_context(tc.tile_pool(name="tpool", bufs=2))
        psum = ctx.enter_context(tc.tile_pool(name="psum", bufs=2, space="PSUM"))
        rpsum = ctx.enter_context(tc.tile_pool(name="rpsum", bufs=2, space="PSUM"))

        xT_sb = singles.tile([128, NK1, T], f32r)
        xT_r = xT.rearrange("(o p) t -> p o t", p=128)
        for xc in range(4):
            nc.sync.dma_start(
                xT_sb[:, xc * (NK1 // 4):(xc + 1) * (NK1 // 4), :],
                xT_r[:, xc * (NK1 // 4):(xc + 1) * (NK1 // 4), :],
            )
        bias_sb = singles.tile([128, NM], f32)
        nc.sync.dma_start(bias_sb, bqd.rearrange("(o p) -> p o", p=128))
        cq_sb = singles.tile([128, T], f32)
        nc.sync.dma_start(cq_sb, cq[:, :])
        sq_sb = singles.tile([128, T], f32)
        nc.sync.dma_start(sq_sb, sq[:, :])
        ck_sb = singles.tile([128, T], f32)
        nc.sync.dma_start(ck_sb, ck[:, :])
        sk_sb = singles.tile([128, T], f32)
        nc.sync.dma_start(sk_sb, sk[:, :])
        rt_sb = singles.tile([128, 128], f32)
        nc.sync.dma_start(rt_sb, rt[:, :])

        zT_sb = singles.tile([128, NM, T], f32r)
        qlT_sb = singles.tile([128, NM, T], f32r)

        def rope_tile(src_ps, c_sb, s_sb, out_tile):
            """out = src*cos + (R @ src)*sin, all [128, T] (2 heads of 64)."""
            src_sb = tpool.tile([128, T], f32, tag="t_src")
            nc.vector.tensor_copy(src_sb, src_ps)
            rot_ps = rpsum.tile([128, T], f32, name="rot_ps")
            # full fp32 matmul: exact for the +-1 rotation matrix
            nc.tensor.matmul(rot_ps, rt_sb, src_sb, start=True, stop=True)
            t_rot = tpool.tile([128, T], f32, tag="t_rot")
            nc.vector.tensor_tensor(t_rot, rot_ps, s_sb, op=ALU.mult)
            t_cos = tpool.tile([128, T], f32, tag="t_cos")
            nc.vector.tensor_tensor(t_cos, src_sb, c_sb, op=ALU.mult)
            nc.vector.tensor_tensor(out_tile, t_cos, t_rot, op=ALU.add)

        # stage 1 (contract over D from xT) and stage 2 (contract over DZ).
        # (weight, k-chunks, src or None=xT, dst sbuf, dram out, bias, rope, scale)
        projs = [
            (wlat, NK1, None, zT_sb, zT_o, False, None, None),
            (wqd, NK1, None, qlT_sb, None, True, None, None),
            (wkr, NK1, None, None, krT_o, False, (ck_sb, sk_sb), None),
            (wqu, NK2, qlT_sb, None, qsT_o, False, None, SCALE),
            (wku, NK2, zT_sb, None, knT_o, False, None, None),
            (wxr, NK2, qlT_sb, None, qrT_o, False, (cq_sb, sq_sb), None),
        ]
        for w_dram, nk, src_sb, dst_sb, out_dram, use_bias, rope, scl in projs:
            if nk == NK2:
                # stage 2: whole [DZ, 1024] weight in two [128, 4, 1024]
                # halves (4 KiB contiguous runs, 2 DMAs instead of 8)
                whs = []
                for half in range(2):
                    wh = wpool.tile([128, NK1, 128], f32r, tag="wt",
                                    name=f"wh{half}")
                    wh4 = wh.rearrange("p (a b) m -> p a (b m)", b=8)
                    nc.sync.dma_start(
                        wh4[:, :4, :],
                        w_dram[half * 512:(half + 1) * 512, :].rearrange(
                            "(ko p) m -> p ko m", p=128
                        ),
                    )
                    whs.append(wh4)
            for mi in range(NM):
                if nk == NK2:
                    ps = psum.tile([128, T], f32, tag="ps_m", name="ps_m")
                    for ki in range(nk):
                        nc.tensor.matmul(
                            ps,
                            whs[ki // 4][:, ki % 4, mi * 128:(mi + 1) * 128],
                            src_sb[:, ki, :],
                            start=(ki == 0),
                            stop=(ki == nk - 1),
                        )
                else:
                    wt = wpool.tile([128, NK1, 128], f32r, tag="wt", name="wt")
                    nc.sync.dma_start(
                        wt,
                        w_dram[:, mi * 128:(mi + 1) * 128].rearrange(
                            "(ks p) m -> p ks m", p=128
                        ),
                    )
                    ps = psum.tile([128, T], f32, tag="ps_m", name="ps_m")
                    for ki in range(nk):
                        nc.tensor.matmul(
                            ps,
                            wt[:, ki, :],
                            xT_sb[:, ki, :],
                            start=(ki == 0),
                            stop=(ki == nk - 1),
                        )
                if dst_sb is not None:
                    stage = dst_sb[:, mi, :]
                else:
                    stage = opool.tile([128, T], f32, tag="stage1")
                if use_bias:
                    nc.vector.tensor_scalar_add(stage, ps, bias_sb[:, mi:mi + 1])
                elif rope is not None:
                    rope_tile(ps, rope[0], rope[1], stage)
                elif scl is not None:
                    nc.vector.tensor_scalar_mul(stage, ps, scl)
                else:
                    nc.vector.tensor_copy(stage, ps)
                if out_dram is not None:
                    nc.sync.dma_start(out_dram[mi * 128:(mi + 1) * 128, :], stage)

        # v = z @ W_v_up, token-major [T, 1024]
        for ch in range(2):
            wv = wpool.tile([128, NK1, 128], f32r, tag="wt", name="wv")
            wv4 = wv.rearrange("p (a b) m -> p a (b m)", b=4)  # [128,8,512] view
            nc.sync.dma_start(
                wv4[:, :NK2, :],
                wvu[:, ch * 512:(ch + 1) * 512].rearrange(
                    "(ko p) m -> p ko m", p=128
                ),
            )
            for ti in range(T // 128):
                ps = psum.tile([128, 512], f32, tag="ps_m", name="ps_v")
                for ki in range(NK2):
                    nc.tensor.matmul(
                        ps,
                        zT_sb[:, ki, ti * 128:(ti + 1) * 128],
                        wv4[:, ki, :],
                        start=(ki == 0),
                        stop=(ki == NK2 - 1),
                    )
                stage = opool.tile([128, 512], f32, tag="stage_v")
                nc.vector.tensor_copy(stage, ps)
                nc.sync.dma_start(
                    v_o[ti * 128:(ti + 1) * 128, ch * 512:(ch + 1) * 512], stage
                )
    nc.compile()
    return nc


# ---------------------------------------------------------------------------
# Launch B: per-core (batch, head-group of 4) -> scores, attn, head_outT.
# ---------------------------------------------------------------------------

def build_launch_b():
    nc = bacc.Bacc("TRN2", target_bir_lowering=False)
    qT = nc.dram_tensor("qT", [512, L], f32r, kind="ExternalInput")
    kT = nc.dram_tensor("kT", [512, L], f32r, kind="ExternalInput")
    vv = nc.dram_tensor("vv", [L, 256], f32r, kind="ExternalInput")
    sc_o = nc.dram_tensor("sc", [4, L, L], f32, kind="ExternalOutput")
    at_o = nc.dram_tensor("at", [4, L, L], f32, kind="ExternalOutput")
    hoT_o = nc.dram_tensor("hoT", [256, L], f32, kind="ExternalOutput")

    NQT = L // 128  # 16 q-tiles of 128
    NQB = L // 512  # 4 q-blocks of 512
    NKC = L // 128  # 16 k-chunks of 128

    with ExitStack() as ctx:
        tc = ctx.enter_context(tile.TileContext(nc))
        singles = ctx.enter_context(tc.tile_pool(name="singles", bufs=1))
        hpool = ctx.enter_context(tc.tile_pool(name="hpool", bufs=2))
        rowpool = ctx.enter_context(tc.tile_pool(name="rowpool", bufs=4))
        stpool = ctx.enter_context(tc.tile_pool(name="stpool", bufs=4))
        spool = ctx.enter_context(tc.tile_pool(name="spool", bufs=8))
        hopool = ctx.enter_context(tc.tile_pool(name="hopool", bufs=2))
        pspool = ctx.enter_context(tc.tile_pool(name="pspool", bufs=3, space="PSUM"))
        ps2pool = ctx.enter_context(tc.tile_pool(name="ps2pool", bufs=2, space="PSUM"))
        opsum = ctx.enter_context(tc.tile_pool(name="opsum", bufs=2, space="PSUM"))

        neg_reg = nc.gpsimd.to_reg(NEG)
        zero_reg = nc.gpsimd.to_reg(0.0)
        m1e6 = singles.tile([128, 512], f32)
        nc.vector.memset(m1e6, NEG)
        m1e6r = singles.tile([128, 512], f32r)
        nc.vector.tensor_copy(m1e6r, m1e6)
        v_sb = singles.tile([128, NKC, 256], f32r)
        nc.sync.dma_start(v_sb, vv.rearrange("(ko p) c -> p ko c", p=128))

        # sT-band mask pairs: for band offset j, sT tile (partition=k, free=q)
        # keeps entries with q >= k: col >= 128*j + p. masked = ps*m + c.
        ones_t = singles.tile([128, 512], f32)
        nc.vector.memset(ones_t, 1.0)
        zeros_t = singles.tile([128, 512], f32)
        nc.vector.memset(zeros_t, 0.0)
        mT = []
        cT = []
        for j in range(4):
            m_j = singles.tile([128, 512], f32, name=f"mT_{j}")
            nc.gpsimd.affine_select(
                m_j, ones_t, pattern=[[1, 512]], base=-128 * j,
                channel_multiplier=-1, compare_op=ALU.is_ge, fill=zero_reg,
            )
            c_j = singles.tile([128, 512], f32, name=f"cT_{j}")
            nc.gpsimd.affine_select(
                c_j, zeros_t, pattern=[[1, 512]], base=-128 * j,
                channel_multiplier=-1, compare_op=ALU.is_ge, fill=neg_reg,
            )
            mT.append(m_j)
            cT.append(c_j)

        def load_head(h):
            q_t = hpool.tile([128, L], f32r, tag="qT_h", name=f"qT_{h}")
            nc.sync.dma_start(q_t, qT[h * 128:(h + 1) * 128, :])
            k_t = hpool.tile([128, L], f32r, tag="kT_h", name=f"kT_{h}")
            nc.sync.dma_start(k_t, kT[h * 128:(h + 1) * 128, :])
            return q_t, k_t

        nxt = load_head(0)
        for h in range(4):
            qT_h, kT_h = nxt
            if h + 1 < 4:
                nxt = load_head(h + 1)  # prefetch before this head's DMAs

            def phase1_row(i, qT_h=qT_h, kT_h=kT_h, h=h):
                # scores row of 128 queries + softmax -> sc/at DMA
                nb = i // 4 + 1  # 512-blocks covering k <= q
                s_row = rowpool.tile([128, L], f32, tag="s_row", name="s_row")
                for kb in range(nb):
                    ps = pspool.tile([128, 512], f32, tag="ps_s", name="ps_s")
                    nc.tensor.matmul(
                        ps,
                        qT_h[:, i * 128:(i + 1) * 128],
                        kT_h[:, kb * 512:(kb + 1) * 512],
                        start=True,
                        stop=True,
                    )
                    blk = s_row[:, kb * 512:(kb + 1) * 512]
                    nc.vector.tensor_copy(blk, ps)
                    if kb == i // 4:  # diagonal block: keep iff q - k >= 0
                        nc.gpsimd.affine_select(
                            blk,
                            blk,
                            pattern=[[-1, 512]],
                            base=i * 128 - kb * 512,
                            channel_multiplier=1,
                            compare_op=ALU.is_ge,
                            fill=neg_reg,
                        )
                nc.sync.dma_start(
                    sc_o[h, i * 128:(i + 1) * 128, 0:nb * 512],
                    s_row[:, :nb * 512],
                )
                e_row = rowpool.tile([128, L], f32, tag="e_row", name="e_row")
                nc.vector.tensor_scalar_max(
                    e_row[:, :nb * 512], s_row[:, :nb * 512], -80.0
                )
                ssum = spool.tile([128, 1], f32, tag="ssum", name="ssum")
                nc.scalar.activation(
                    e_row[:, :nb * 512],
                    e_row[:, :nb * 512],
                    AF.Exp,
                    accum_out=ssum,
                )
                sinv = spool.tile([128, 1], f32, tag="sinv", name="sinv")
                nc.vector.reciprocal(sinv, ssum)
                nc.vector.tensor_scalar_mul(
                    e_row[:, :nb * 512], e_row[:, :nb * 512], sinv
                )
                return i, e_row, nb

            ho_row = hopool.tile([64, L], f32, tag="ho", name="ho_row")

            def phase2_block(qb, qT_h=qT_h, kT_h=kT_h, h=h, ho_row=ho_row):
                # head_outT q-block = (masked scores)^T-path @ v
                po = opsum.tile([64, 512], f32, tag="po", name="po")
                for kc in range(NKC):
                    if kc <= 4 * qb + 3:
                        ps2 = ps2pool.tile([128, 512], f32, tag="ps_st",
                                           name="ps_st")
                        nc.tensor.matmul(
                            ps2,
                            kT_h[:, kc * 128:(kc + 1) * 128],
                            qT_h[:, qb * 512:(qb + 1) * 512],
                            start=True,
                            stop=True,
                        )
                        sT = stpool.tile([128, 512], f32r, tag="sT", name="sT")
                        if kc >= 4 * qb:  # diagonal band: mask = ps*m + c
                            j = kc - 4 * qb
                            nc.vector.tensor_tensor(sT, ps2, mT[j], op=ALU.mult)
                            nc.vector.tensor_tensor(sT, sT, cT[j], op=ALU.add)
                        else:
                            nc.vector.tensor_copy(sT, ps2)
                        rhs = sT
                    else:
                        rhs = m1e6r
                    nc.tensor.matmul(
                        po,
                        v_sb[:, kc, h * 64:(h + 1) * 64],
                        rhs,
                        start=(kc == 0),
                        stop=(kc == NKC - 1),
                    )
                nc.vector.tensor_copy(
                    ho_row[:, qb * 512:(qb + 1) * 512], po
                )
                if qb == NQB - 1:
                    nc.sync.dma_start(hoT_o[h * 64:(h + 1) * 64, :], ho_row)

            def at_dma(pend, h=h):
                i, e_row, nb = pend
                nc.sync.dma_start(
                    at_o[h, i * 128:(i + 1) * 128, 0:nb * 512],
                    e_row[:, :nb * 512],
                )

            # interleave so phase-2 PE work overlaps phase-1 DMA drains;
            # lag each attn DMA one row so a pending exp-chain never
            # head-of-line-blocks the next ready scores DMA
            import os as _os
            _abl = _os.environ.get("B_ABLATE", "")
            pend = None
            for i in range(NQT):
                cur = phase1_row(i)
                if pend is not None and "noat" not in _abl:
                    at_dma(pend)
                pend = cur
                if i % 4 == 3 and "nop2" not in _abl:
                    phase2_block(i // 4)
            if "noat" not in _abl:
                at_dma(pend)
    nc.compile()
    return nc


# ---------------------------------------------------------------------------
# Launch C: per-core token shard -> out = head_out @ W_o.
# ---------------------------------------------------------------------------

def build_launch_c():
    nc = bacc.Bacc("TRN2", target_bir_lowering=False)
    hoT = nc.dram_tensor("hoT", [H * HD, T], f32r, kind="ExternalInput")
    wo = nc.dram_tensor("wo", [H * HD, D], f32r, kind="ExternalInput")
    out_o = nc.dram_tensor("out", [T, D], f32, kind="ExternalOutput")

    NK = (H * HD) // 128  # 8
    with ExitStack() as ctx:
        tc = ctx.enter_context(tile.TileContext(nc))
        singles = ctx.enter_context(tc.tile_pool(name="singles", bufs=1))
        wpool = ctx.enter_context(tc.tile_pool(name="wpool", bufs=3))
        opool = ctx.enter_context(tc.tile_pool(name="opool", bufs=4))
        psum = ctx.enter_context(tc.tile_pool(name="psum", bufs=4, space="PSUM"))

        hoT_sb = singles.tile([128, NK, T], f32r)
        nc.sync.dma_start(hoT_sb, hoT.rearrange("(o p) t -> p o t", p=128))

        for nb in range(D // 512):
            wt = wpool.tile([128, NK, 512], f32r, tag="wt", name="wt")
            nc.sync.dma_start(
                wt,
                wo[:, nb * 512:(nb + 1) * 512].rearrange(
                    "(o p) d -> p o d", p=128
                ),
            )
            for ti in range(T // 128):
                ps = psum.tile([128, 512], f32, tag="ps_c")
                for ki in range(NK):
                    nc.tensor.matmul(
                        ps,
                        hoT_sb[:, ki, ti * 128:(ti + 1) * 128],
                        wt[:, ki, :],
                        start=(ki == 0),
                        stop=(ki == NK - 1),
                    )
                stage = opool.tile([128, 512], f32, tag="stage_c")
                nc.vector.tensor_copy(stage, ps)
                nc.sync.dma_start(
                    out_o[ti * 128:(ti + 1) * 128, nb * 512:(nb + 1) * 512],
                    stage,
                )
    nc.compile()
    return nc


# ---------------------------------------------------------------------------
# Host orchestration.
# ---------------------------------------------------------------------------

def _rope_tables(l0, scale):
    """[128, T] cos/sin tables mimicking the reference fp32 computation."""
    invf = (
        np.float32(1.0)
        / np.float32(10000.0)
        ** (np.arange(0, HD, 2, dtype=np.float32) / np.float32(HD))
    )  # [32]
    pos = np.arange(l0, l0 + T, dtype=np.float32)  # [T]
    p = np.arange(128)
    ang = pos[None, :] * invf[p % 32][:, None]  # [128, T], fp32 product
    ang = ang.astype(np.float32)
    c = np.cos(ang).astype(np.float32) * np.float32(scale)
    s = np.sin(ang).astype(np.float32) * np.float32(scale)
    return np.ascontiguousarray(c), np.ascontiguousarray(s)


def _rotmatT():
    r = np.zeros((128, 128), dtype=np.float32)
    for h0 in (0, 64):
        for i in range(32):
            r[h0 + i, h0 + 32 + i] = -1.0  # out[i] -= in[i+32]
            r[h0 + 32 + i, h0 + i] = 1.0  # out[i+32] += in[i]
    return np.ascontiguousarray(r.T)


_BUILT = {}


def _get(name, builder):
    if name not in _BUILT:
        _BUILT[name] = builder()
    return _BUILT[name]


_EXEC_NS = []  # per-launch exec_time_ns (if traced) of the most recent call
_WALL_S = []  # per-launch wall seconds of the most recent kernel() call


def _run(nc, in_maps, trace):
    import time as _time

    from concourse.bass_utils import run_bass_kernel_spmd

    t0 = _time.perf_counter()
    res = run_bass_kernel_spmd(
        nc, in_maps, core_ids=list(range(NC)), trace=trace
    )
    _WALL_S.append(_time.perf_counter() - t0)
    _EXEC_NS.append(res.exec_time_ns)
    return res.results


def kernel(**inputs):
    trace = bool(int(os.environ.get("KERNEL_TRACE", "0")))
    _EXEC_NS.clear()
    _WALL_S.clear()

    x = np.asarray(inputs["x"], dtype=np.float32)
    wlat = np.ascontiguousarray(np.asarray(inputs["W_latent"], np.float32))
    wqd = np.ascontiguousarray(np.asarray(inputs["W_q_down"], np.float32))
    bqd = np.ascontiguousarray(np.asarray(inputs["b_q_down"], np.float32))
    wqu = np.ascontiguousarray(np.asarray(inputs["W_q_up"], np.float32))
    wku = np.ascontiguousarray(np.asarray(inputs["W_k_up"], np.float32))
    wvu = np.ascontiguousarray(np.asarray(inputs["W_v_up"], np.float32))
    wxr = np.ascontiguousarray(np.asarray(inputs["W_x_rope"], np.float32))
    wkr = np.ascontiguousarray(np.asarray(inputs["W_k_rope"], np.float32))
    wo = np.ascontiguousarray(np.asarray(inputs["W_o"], np.float32))

    xT = np.ascontiguousarray(x.reshape(B * L, D).T)  # [D, B*L]
    rt = _rotmatT()

    # ---- launch A ----
    nc_a = _get("A", build_launch_a)
    in_maps = []
    for c in range(NC):
        l0 = (c * T) % L  # position within the batch
        cqt, sqt = _rope_tables(l0, SCALE)
        ckt, skt = _rope_tables(l0, 1.0)
        in_maps.append({
            "xT": np.ascontiguousarray(xT[:, c * T:(c + 1) * T]),
            "wlat": wlat, "wqd": wqd, "bqd": bqd,
            "wqu": wqu, "wku": wku, "wvu": wvu, "wxr": wxr, "wkr": wkr,
            "cq": cqt, "sq": sqt, "ck": ckt, "sk": skt, "rt": rt,
        })
    res_a = _run(nc_a, in_maps, trace)

    def gatherT(name):
        # per-core [C, T] -> per-batch [C, L]
        return [
            np.concatenate([res_a[4 * b + j][name] for j in range(4)], axis=1)
            for b in range(B)
        ]

    zT = gatherT("zT")
    qsT = gatherT("qsT")
    knT = gatherT("knT")
    qrT = gatherT("qrT")
    krT = gatherT("krT")
    v_full = [
        np.concatenate([res_a[4 * b + j]["v"] for j in range(4)], axis=0)
        for b in range(B)
    ]  # [L, 1024] per batch

    z = np.stack([zT[b].T for b in range(B)])  # (B, L, DZ)
    key_rope = np.stack([krT[b].T for b in range(B)])  # (B, L, H*HD)

    # ---- launch B ----
    nc_b = _get("B", build_launch_b)
    in_maps = []
    for c in range(NC):
        b, g = divmod(c, 4)
        if g < 2:  # heads 0-7: nope channels
            q_in = qsT[b][512 * g:512 * (g + 1), :]
            k_in = knT[b][512 * g:512 * (g + 1), :]
        else:  # heads 8-15: rope channels
            q_in = qrT[b][512 * (g - 2):512 * (g - 1), :]
            k_in = krT[b][512 * (g - 2):512 * (g - 1), :]
        in_maps.append({
            "qT": np.ascontiguousarray(q_in),
            "kT": np.ascontiguousarray(k_in),
            "vv": np.ascontiguousarray(v_full[b][:, 256 * g:256 * (g + 1)]),
        })
    res_b = _run(nc_b, in_maps, trace)

    scores = np.empty((B, H, L, L), dtype=np.float32)
    attn = np.zeros((B, H, L, L), dtype=np.float32)
    hoT = [np.empty((H * HD, L), dtype=np.float32) for _ in range(B)]
    for c in range(NC):
        b, g = divmod(c, 4)
        scores[b, 4 * g:4 * (g + 1)] = res_b[c]["sc"]
        attn[b, 4 * g:4 * (g + 1)] = res_b[c]["at"]
        hoT[b][256 * g:256 * (g + 1), :] = res_b[c]["hoT"]
    # masked blocks beyond each q-tile's covered range were never written on
    # device: attn is exactly 0 there (zero output buffer); scores get the
    # causal -1e6 fill here.
    for i in range(L // 128):
        k0 = (i // 4 + 1) * 512
        if k0 < L:
            scores[:, :, i * 128:(i + 1) * 128, k0:] = np.float32(NEG)
            attn[:, :, i * 128:(i + 1) * 128, k0:] = 0.0

    # ---- launch C ----
    nc_c = _get("C", build_launch_c)
    in_maps = []
    for c in range(NC):
        b = c // 4
        t0 = (c % 4) * T
        in_maps.append({
            "hoT": np.ascontiguousarray(hoT[b][:, t0:t0 + T]),
            "wo": wo,
        })
    res_c = _run(nc_c, in_maps, trace)
    out = np.empty((B, L, D), dtype=np.float32)
    for c in range(NC):
        b = c // 4
        t0 = (c % 4) * T
        out[b, t0:t0 + T, :] = res_c[c]["out"]

    return out, z, key_rope, attn, scores
